# revision 10
# baseline (speedup 1.0000x reference)
"""Trainium2 Bass kernel for nn_ChainModel (neural 1-form chain classifier).

Computation (per edge e, graph b):
    mid = 0.5*(p0+p1); t = p1-p0
    h = relu(mid @ W1 + b1)                      [64]
    omega = (h @ W2 + b2) viewed [32, 16]
    X[c] = omega[c, :] . t                       [32]
    feats[b, c] = sum_{e in b} X[c]^2
    BN(train-stats) -> 3-layer classifier -> log_softmax
Host: fold run sums into per-graph feats, BN + classifier + log_softmax (f64).

v2 "c-major" device pipeline (no DMA transposes, no DVE reduction tree):
    B:  PE matmul  wfh[33,65] stationary, chains chunk moving
        -> h' c-major [65, e] f32 PSUM (row 64 = relu(1) = 1)
    C:  ACT relu -> h_cm fp16 SBUF [65, e]
    D:  PE matmul (x4 cd-tiles) w2p[65,128g] stationary, h_cm moving
        -> omT [(c%8)*16+d, e] f32 PSUM; b2 folded via h' ones row
    E:  psi = omT * t_rep  (t_rep[p,e] = t[e, p%16] host-prepped, DMA'd)
        path per chunk-pair: D: DVE direct from PSUM f32 (1x)
                             A: ACT evac fp16 SBUF, then DVE mul (2x)
                             P: Pool (gpsimd) direct from PSUM
    F:  PE matmul (x4) psiT[128,128] stationary, S[128,8] moving
        -> X[e, 32] f32 PSUM  (S[p,j] = 1 iff p//16 == j: d-reduce on PE)
    G:  ACT square X -> x2 fp16 SBUF
    H:  PE matmul run-indicators[128,R] stationary, x2[128,32] moving
        -> per-run sums f32 PSUM -> evac -> DMA out

Sharding: contiguous edge ranges, E/8 per core; graphs straddling core or
chunk boundaries are handled by the host-side run->graph accumulation.
"""

import numpy as np
from contextlib import ExitStack

# ---------------- problem constants (hardcoded per contest rules) -----------
E_TOT = 524288
B_GRAPHS = 256
DIN = 16
HID = 64
C_STEPS = 32
NCLS = 10
EPS = 1e-5
NCORES = 8
EC = E_TOT // NCORES          # 65536 edges per core
CHUNK = 128                   # edges per chunk
NCHUNK = EC // CHUNK          # 512
KAUG = 2 * DIN + 1            # 33: chains dims + ones row
HA = HID + 1                  # 65: h rows + ones row (for b2 fold)

_BUILD_CACHE = {}


# tuning knobs
CFG = {
    "super": 16,        # chunks per super-tile
    "pb": 2,            # chunks per om/psi tile (pair-batched)
    "rb": 4,            # chunks per B/relu group
    "xb": 16,           # chunks per X/square group
    # psi engine schedule per chunk-pair, cycled:
    #   D = DVE direct from PSUM f32 (1x)
    #   Q = ACT evac fp16 + Pool mul
    #   A = ACT evac fp16 + DVE mul (2x)
    "pattern": "DDQ",
    "om_bufs": 2,
    "psi_bufs": 4,
    "h_bufs": 3,
    "x_bufs": 1,
    "rps_bufs": 1,
    "rsb_engine": "act",
}


# ============================ device program ================================
def _build_program(runs_per_chunk, cfg=None):
    import concourse.bacc as bacc
    import concourse.mybir as mybir
    import concourse.tile as tile

    dt = mybir.dt
    AF = mybir.ActivationFunctionType
    ALU = mybir.AluOpType
    R = runs_per_chunk
    cfg = dict(CFG, **(cfg or {}))
    SUPER = cfg["super"]
    NSUPER = NCHUNK // SUPER
    PB = cfg["pb"]
    RB = cfg["rb"]
    XB = cfg["xb"]
    pattern = cfg["pattern"]

    nc = bacc.Bacc("TRN2", target_bir_lowering=False, debug=False,
                   num_devices=NCORES)

    # DRAM I/O
    d_chains = nc.dram_tensor("chainsT", [KAUG, EC], dt.float16,
                              kind="ExternalInput").ap()
    d_trep = nc.dram_tensor("trep", [128, EC], dt.float16,
                            kind="ExternalInput").ap()
    d_wfh = nc.dram_tensor("wfh", [KAUG, HA], dt.float16,
                           kind="ExternalInput").ap()
    d_w2p = nc.dram_tensor("w2p", [HA, C_STEPS * DIN], dt.float16,
                           kind="ExternalInput").ap()
    d_sred = nc.dram_tensor("sred", [128, 8], dt.float16,
                            kind="ExternalInput").ap()
    d_ind = nc.dram_tensor("ind", [128, NCHUNK * R], dt.float16,
                           kind="ExternalInput").ap()
    d_out = nc.dram_tensor("runsums", [R, NCHUNK * C_STEPS], dt.float32,
                           kind="ExternalOutput").ap()

    with tile.TileContext(nc) as tc, ExitStack() as ctx:
        const_pool = ctx.enter_context(tc.tile_pool(name="const", bufs=1))
        sb_in = ctx.enter_context(tc.tile_pool(name="sbin", bufs=2))
        sb_h = ctx.enter_context(tc.tile_pool(name="sbh", bufs=cfg["h_bufs"]))
        sb_psi = ctx.enter_context(
            tc.tile_pool(name="sbpsi", bufs=cfg["psi_bufs"]))
        sb_oms = ctx.enter_context(tc.tile_pool(name="sboms", bufs=2))
        sb_x2 = ctx.enter_context(tc.tile_pool(name="sbx2", bufs=2))
        sb_r = ctx.enter_context(tc.tile_pool(name="sbr", bufs=2))
        # PSUM slots are bank-aligned and per-tag: budget is
        # bp 2x1 + om 2x2 + xps 1x1 + rps 1x1 = 8 banks
        ps_b = ctx.enter_context(tc.tile_pool(name="psb", bufs=2,
                                              space="PSUM"))
        ps_o = ctx.enter_context(tc.tile_pool(name="pso", bufs=cfg["om_bufs"],
                                              space="PSUM"))
        ps_x = ctx.enter_context(tc.tile_pool(name="psx", bufs=cfg["x_bufs"],
                                              space="PSUM"))
        ps_r = ctx.enter_context(tc.tile_pool(name="psr",
                                              bufs=cfg["rps_bufs"],
                                              space="PSUM"))

        # constants loaded once
        wfh = const_pool.tile([KAUG, HA], dt.float16)
        nc.sync.dma_start(wfh[:], d_wfh)
        w2p = const_pool.tile([HA, C_STEPS * DIN], dt.float16)
        nc.sync.dma_start(w2p[:], d_w2p)
        sred = const_pool.tile([128, 8], dt.float16)
        nc.sync.dma_start(sred[:], d_sred)
        ind = const_pool.tile([128, NCHUNK * R], dt.float16)
        nc.sync.dma_start(ind[:], d_ind)

        for s in range(NSUPER):
            ch = sb_in.tile([KAUG, SUPER * CHUNK], dt.float16, tag="ch")
            nc.sync.dma_start(ch[:], d_chains[:, s * SUPER * CHUNK:
                                             (s + 1) * SUPER * CHUNK])
            trep = sb_in.tile([128, SUPER, CHUNK], dt.float16, tag="trep")
            nc.sync.dma_start(trep[:], d_trep[:, s * SUPER * CHUNK:
                                              (s + 1) * SUPER * CHUNK])
            rps = ps_r.tile([R, SUPER * C_STEPS], dt.float32, tag="rps")
            xps = None

            for q in range(SUPER // RB):
                # ---- B + C: h' c-major, relu ----
                bp = ps_b.tile([HA, RB, CHUNK], dt.float32, tag="bp")
                for j in range(RB):
                    k = q * RB + j
                    nc.tensor.matmul(bp[:, j, :], wfh[:],
                                     ch[:, k * CHUNK:(k + 1) * CHUNK],
                                     start=True, stop=True)
                h4 = sb_h.tile([HA, RB, CHUNK], dt.float16, tag="h4")
                nc.scalar.activation(h4[:], bp[:], AF.Relu)

                for kpin in range(RB // PB):
                    kp = q * (RB // PB) + kpin
                    path = pattern[(s * (SUPER // PB) + kp) % len(pattern)]
                    # ---- D: omT c-major ----
                    om = ps_o.tile([128, PB, 4, CHUNK], dt.float32, tag="om")
                    for j in range(PB):
                        k = kp * PB + j
                        h_k = h4[:, kpin * PB + j, :]
                        for g in range(4):
                            nc.tensor.matmul(om[:, j, g, :],
                                             w2p[:, g * 128:(g + 1) * 128],
                                             h_k, start=True, stop=True)
                    # ---- E: psi = omT * t_rep ----
                    t_b = trep[:, kp * PB:(kp + 1) * PB, :].unsqueeze(2) \
                        .broadcast_to([128, PB, 4, CHUNK])
                    psi = sb_psi.tile([128, PB, 4, CHUNK], dt.float16,
                                      tag="psi")
                    if path in ("A", "Q"):
                        # Pool can't touch PSUM: ACT evacuates omega to fp16
                        # SBUF, then Pool (Q) or DVE 2x (A) multiplies
                        oms = sb_oms.tile([128, PB, 4, CHUNK], dt.float16,
                                          tag="oms")
                        nc.scalar.activation(oms[:], om[:], AF.Copy)
                        eng = nc.gpsimd if path == "Q" else nc.vector
                        eng.tensor_tensor(psi[:], oms[:], t_b, ALU.mult)
                    else:
                        nc.vector.tensor_tensor(psi[:], om[:], t_b, ALU.mult)

                    # ---- F: X via S-matmuls; G/H per XB group ----
                    for j in range(PB):
                        k = kp * PB + j
                        if k % XB == 0:
                            xps = ps_x.tile([128, XB, C_STEPS], dt.float32,
                                            tag="xps")
                        for g in range(4):
                            nc.tensor.matmul(
                                xps[:, k % XB, g * 8:(g + 1) * 8],
                                psi[:, j, g, :], sred[:],
                                start=True, stop=True)
                        if k % XB == XB - 1:
                            x2 = sb_x2.tile([128, XB, C_STEPS], dt.float16,
                                            tag="x2")
                            nc.scalar.activation(x2[:], xps[:], AF.Square)
                            for jj in range(XB):
                                kk = (k // XB) * XB + jj
                                gk = s * SUPER + kk
                                nc.tensor.matmul(
                                    rps[:, kk * C_STEPS:(kk + 1) * C_STEPS],
                                    ind[:, gk * R:(gk + 1) * R],
                                    x2[:, jj, :],
                                    start=True, stop=True)

            rsb = sb_r.tile([R, SUPER * C_STEPS], dt.float32, tag="rsb")
            if cfg["rsb_engine"] == "act":
                nc.scalar.activation(rsb[:], rps[:], AF.Copy)
            else:
                nc.vector.tensor_copy(rsb[:], rps[:])
            nc.sync.dma_start(
                d_out[:, s * SUPER * C_STEPS:(s + 1) * SUPER * C_STEPS],
                rsb[:])

    nc.finalize()
    return nc


def _get_program(runs_per_chunk, cfg=None):
    key = (runs_per_chunk, tuple(sorted((cfg or {}).items())))
    if key not in _BUILD_CACHE:
        _BUILD_CACHE[key] = _build_program(runs_per_chunk, cfg)
    return _BUILD_CACHE[key]


# ============================ host-side glue ================================
def _host_prep_weights(W1, b1, W2, b2):
    """wfh [33,65] (mid-fold W1 | ones col); w2p [65,512] (W2 | b2 row)."""
    wfh = np.zeros((KAUG, HA), np.float64)
    wfh[0:DIN, 0:HID] = 0.5 * W1
    wfh[DIN:2 * DIN, 0:HID] = 0.5 * W1
    wfh[2 * DIN, 0:HID] = b1          # ones-row -> +b1
    wfh[2 * DIN, HID] = 1.0           # h' row 64 = relu(1) = 1
    w2p = np.empty((HA, C_STEPS * DIN), np.float64)
    w2p[0:HID] = W2
    w2p[HID] = b2
    return wfh.astype(np.float16), w2p.astype(np.float16)


def _host_prep_core(chains_core):
    """chains slice [EC, 2, 16] -> chainsT_aug [33, EC] fp16."""
    flat = np.ascontiguousarray(chains_core.reshape(EC, 2 * DIN).T)  # [32, EC]
    out = np.empty((KAUG, EC), np.float16)
    out[0:2 * DIN] = flat.astype(np.float16)
    out[2 * DIN] = 1.0
    return out


def _host_prep_trep(chains_core):
    """t_rep [128, EC]: row p = t[:, p % 16] = (p1-p0)[:, p % 16]."""
    t = (chains_core[:, 1, :] - chains_core[:, 0, :]).astype(np.float16)
    return np.ascontiguousarray(np.tile(t.T, (8, 1)))     # [128, EC]


def _host_sred():
    return np.repeat(np.eye(8, dtype=np.float16), 16, axis=0)  # [128, 8]


def _host_runs(edge_slices):
    """Per-core run indicators + run->graph map.

    Returns R, and per core: ind [128, NCHUNK*R] fp16,
    run2graph [NCHUNK, R] int32 (-1 = unused).
    """
    es = np.asarray(edge_slices, np.int64)
    seg = np.searchsorted(es, np.arange(E_TOT), side="right") - 1
    seg = np.clip(seg, 0, B_GRAPHS - 1).astype(np.int32)

    segc = seg.reshape(NCORES * NCHUNK, CHUNK)
    nruns = 1 + (np.diff(segc, axis=1) != 0).sum(axis=1)
    runs_needed = int(nruns.max())
    R = max(4, int(2 ** np.ceil(np.log2(runs_needed))))

    inds, maps = [], []
    for core in range(NCORES):
        ind = np.zeros((128, NCHUNK * R), np.float16)
        r2g = np.full((NCHUNK, R), -1, np.int32)
        sc = seg[core * EC:(core + 1) * EC].reshape(NCHUNK, CHUNK)
        for k in range(NCHUNK):
            g = sc[k]
            bnd = np.flatnonzero(np.diff(g)) + 1
            starts = np.concatenate(([0], bnd))
            ends = np.concatenate((bnd, [CHUNK]))
            for r, (a, b) in enumerate(zip(starts, ends)):
                ind[a:b, k * R + r] = 1.0
                r2g[k, r] = g[a]
        inds.append(ind)
        maps.append(r2g)
    return R, inds, maps


def _host_tail(feats, gamma, beta, C1w, C1b, C2w, C2b, C3w, C3b):
    f = feats.astype(np.float64)
    mean = f.mean(axis=0)
    var = f.var(axis=0)
    f = (f - mean) / np.sqrt(var + EPS) * gamma.astype(np.float64) \
        + beta.astype(np.float64)
    h1 = np.maximum(f @ C1w.astype(np.float64) + C1b, 0.0)
    h2 = np.maximum(h1 @ C2w.astype(np.float64) + C2b, 0.0)
    logits = h2 @ C3w.astype(np.float64) + C3b
    mx = logits.max(axis=1, keepdims=True)
    lse = np.log(np.exp(logits - mx).sum(axis=1, keepdims=True)) + mx
    return (logits - lse).astype(np.float32)


def kernel(chains, W1, b1, W2, b2, gamma, beta,
           C1w, C1b, C2w, C2b, C3w, C3b, edge_slices,
           _trace=False):
    import concourse.bass_utils as bass_utils

    chains = np.asarray(chains, np.float32)
    R, inds, r2g = _host_runs(edge_slices)
    wfh, w2p = _host_prep_weights(np.asarray(W1, np.float64),
                                  np.asarray(b1, np.float64),
                                  np.asarray(W2, np.float64),
                                  np.asarray(b2, np.float64))
    sred = _host_sred()

    nc = _get_program(R)
    in_maps = []
    for core in range(NCORES):
        sl = chains[core * EC:(core + 1) * EC]
        in_maps.append({
            "chainsT": _host_prep_core(sl),
            "trep": _host_prep_trep(sl),
            "wfh": wfh,
            "w2p": w2p,
            "sred": sred,
            "ind": inds[core],
        })

    import time as _time
    _t0 = _time.time()
    try:
        res = bass_utils.run_bass_kernel_spmd(
            nc, in_maps, core_ids=list(range(NCORES)), trace=_trace)
    except Exception:
        # transient device errors (e.g. NRT_EXEC_UNIT_UNRECOVERABLE left by
        # a previous crashed process) usually clear on the next attempt
        _time.sleep(2.0)
        res = bass_utils.run_bass_kernel_spmd(
            nc, in_maps, core_ids=list(range(NCORES)), trace=_trace)
    kernel._last_run_s = _time.time() - _t0

    # fold run sums into per-graph feats
    feats = np.zeros((B_GRAPHS, C_STEPS), np.float64)
    for core in range(NCORES):
        rs = res.results[core]["runsums"].astype(np.float64)
        rs = rs.reshape(R, NCHUNK, C_STEPS)          # [r, chunk, c]
        m = r2g[core]                                 # [chunk, R]
        valid = m >= 0
        np.add.at(feats, m[valid],
                  np.transpose(rs, (1, 0, 2))[valid])

    out = _host_tail(feats, np.asarray(gamma), np.asarray(beta),
                     np.asarray(C1w), np.asarray(C1b),
                     np.asarray(C2w), np.asarray(C2b),
                     np.asarray(C3w), np.asarray(C3b))
    kernel._last_exec_ns = res.exec_time_ns
    return out
